# revision 25
# baseline (speedup 1.0000x reference)
"""AttentionBlock (BatchNorm + 8-head self-attention + proj + residual) on 8 TRN2
NeuronCores, data-parallel over the batch of 8 images.

Reference computation (per batch of 8 images, c=512, T=32*32=1024):
  xf = x.reshape(b, c, T)
  mean/var over (b, T) per channel  (BatchNorm1d training mode, biased var)
  xn = (xf - mean) * rsqrt(var + eps) * bn_w + bn_b
  qkv = qkv_w @ xn + qkv_b          (per image; heads interleaved 192-row blocks)
  per head: softmax((q*s)^T (k*s)) @ v^T   with s = ch^-0.25
  out = xf + proj_w @ h + proj_b

Sharding: each core computes ONE image end-to-end. The only cross-image
coupling is the BN statistics; each core redundantly computes the global
stats from the full x (streamed tile-by-tile through bn_stats/bn_aggr).
Each core's input "xall" is rolled so its own image is xall[0] (stats are
order-invariant), keeping the program SPMD-identical across cores.

Matmuls run in float32r (full fp32 storage; PE rounds to a reduced internal
mantissa) which streams at 1 column/cycle for N>=256 -- 4x faster than fp32
with ~1e-5 relative error (measured on hw).
"""

import numpy as np

import concourse.bass as bass
import concourse.mybir as mybir
from concourse.bass_utils import run_bass_kernel_spmd
from concourse.tile import TileContext

F32 = mybir.dt.float32
F32R = mybir.dt.float32r
AF = mybir.ActivationFunctionType
OP = mybir.AluOpType

B = 8          # batch (== n cores)
C = 512        # channels
T = 1024       # sequence length (32*32)
NH = 8         # heads
CH = C // NH   # head dim = 64
CT = C // 128  # channel partition tiles = 4
EPS = 1e-5
N_CORES = 8


def split_sync_waits(nc, max_waits=1):
    """This walrus build rejects instructions carrying more than one sem wait
    ("Too many sync wait commands"); move overflow waits onto preceding NoOps
    on the same engine (per-engine program order preserves semantics)."""
    n_new = 0
    for f in nc.m.functions:
        for blk in f.blocks:
            insts = blk.instructions
            i = 0
            while i < len(insts):
                ins = insts[i]
                si = ins.sync_info
                if si is not None and len(si.on_wait) > max_waits:
                    waits = list(si.on_wait)
                    keep, rest = waits[:max_waits], waits[max_waits:]
                    ins.sync_info = type(si)(on_wait=keep, on_update=list(si.on_update))
                    while rest:
                        chunk, rest = rest[:max_waits], rest[max_waits:]
                        nop = mybir.InstNoOp(name=f"waitsplit_{n_new}", ins=[], outs=[])
                        n_new += 1
                        nop.engine = ins.engine
                        nop.sync_info = type(si)(on_wait=chunk, on_update=[])
                        insts.insert(i, nop)
                        i += 1
                i += 1
    return n_new


def build_nc(split=True):
    nc = bass.Bass()

    xall = nc.declare_dram_parameter("xall", [B, C, T], F32, isOutput=False)
    wqkT = nc.declare_dram_parameter("wqkT", [C, 2 * C], F32, isOutput=False)
    wvT = nc.declare_dram_parameter("wvT", [C, C], F32, isOutput=False)
    pwT = nc.declare_dram_parameter("pwT", [C, C], F32, isOutput=False)
    qb = nc.declare_dram_parameter("qb", [128, CT], F32, isOutput=False)
    kb = nc.declare_dram_parameter("kb", [128, CT], F32, isOutput=False)
    vbb = nc.declare_dram_parameter("vbb", [128, C], F32, isOutput=False)
    pb = nc.declare_dram_parameter("pb", [128, CT], F32, isOutput=False)
    bnw = nc.declare_dram_parameter("bnw", [128, CT], F32, isOutput=False)
    bnb = nc.declare_dram_parameter("bnb", [128, CT], F32, isOutput=False)
    bsel = nc.declare_dram_parameter("bsel", [40, 2 * CT, 128], F32, isOutput=False)
    out = nc.declare_dram_parameter("out", [C, T], F32, isOutput=True)

    with TileContext(nc) as tc:
        with (
            tc.tile_pool(name="persist", bufs=1) as pp,
            tc.tile_pool(name="stage", bufs=3) as sp,
            tc.tile_pool(name="epool", bufs=2) as ep,
            tc.tile_pool(name="small", bufs=2) as smp,
            tc.tile_pool(name="opool", bufs=2) as op_pool,
            tc.tile_pool(name="psA", bufs=4, space="PSUM") as psA,
            tc.tile_pool(name="psW", bufs=2, space="PSUM") as psW,
        ):
            # ---- constants -------------------------------------------------
            qb_t = pp.tile([128, CT], F32, tag="qb")
            kb_t = pp.tile([128, CT], F32, tag="kb")
            vbb_t = pp.tile([128, C], F32, tag="vbb")
            pb_t = pp.tile([128, CT], F32, tag="pb")
            bnw_t = pp.tile([128, CT], F32, tag="bnw")
            bnb_t = pp.tile([128, CT], F32, tag="bnb")

            # ---- phase S: BN statistics over all 8 images ------------------
            # Emitted before weight loads so the 16 MiB stream starts at t=0;
            # two images per DMA via a strided 3D access pattern.
            stat = [pp.tile([128, 12 * B], F32, tag=f"stat{k}", name=f"stat{k}") for k in range(CT)]
            for j in range(0, B, 2):
                for k in range(CT):
                    xt = sp.tile([128, 2, T], F32, tag="xstat", bufs=2)
                    nc.sync.dma_start(
                        xt[:], xall[j:j + 2, 128 * k:128 * (k + 1), :]
                        .rearrange("j p t -> p j t"))
                    for jj in range(2):
                        nc.vector.bn_stats(stat[k][:, 12 * (j + jj):12 * (j + jj) + 6],
                                           xt[:, jj, 0:512])
                        nc.vector.bn_stats(stat[k][:, 12 * (j + jj) + 6:12 * (j + jj) + 12],
                                           xt[:, jj, 512:1024])

            nc.sync.dma_start(qb_t[:], qb[:])
            nc.sync.dma_start(kb_t[:], kb[:])
            nc.sync.dma_start(vbb_t[:], vbb[:])
            nc.sync.dma_start(pb_t[:], pb[:])
            nc.sync.dma_start(bnw_t[:], bnw[:])
            nc.sync.dma_start(bnb_t[:], bnb[:])
            ones64 = pp.tile([1, CH], F32R, tag="ones64")
            nc.vector.tensor_scalar(ones64[:], vbb_t[0:1, 0:CH], 0.0, 1.0,
                                    op0=OP.mult, op1=OP.add)

            # ---- weights: DMA then round to f32r on gpsimd -----------------
            wqk_r = [pp.tile([128, 2 * C], F32R, tag=f"wqk{k}", name=f"wqk{k}") for k in range(CT)]
            wv_r = [pp.tile([128, C], F32R, tag=f"wv{k}", name=f"wv{k}") for k in range(CT)]
            pw_r = [pp.tile([128, C], F32R, tag=f"pw{k}", name=f"pw{k}") for k in range(CT)]
            for k in range(CT):
                st = sp.tile([128, T], F32, tag="wst", bufs=2, name="wst1")
                nc.sync.dma_start(st[:], wqkT[128 * k:128 * (k + 1), :])
                nc.vector.tensor_copy(wqk_r[k][:], st[:])
            for k in range(CT):
                st = sp.tile([128, T], F32, tag="wst", bufs=2, name="wst2")
                nc.sync.dma_start(st[:, 0:C], wvT[128 * k:128 * (k + 1), :])
                nc.sync.dma_start(st[:, C:T], pwT[128 * k:128 * (k + 1), :])
                nc.gpsimd.tensor_copy(wv_r[k][:], st[:, 0:C])
                nc.gpsimd.tensor_copy(pw_r[k][:], st[:, C:T])

            bsel_r = pp.tile([40, 2 * CT, 128], F32R, tag="bsel_r")
            bsel_st = sp.tile([128, T], F32, tag="wst", bufs=2, name="bselst")
            nc.sync.dma_start(bsel_st[0:40, :], bsel[:].rearrange("p a b -> p (a b)"))
            nc.vector.tensor_copy(
                bsel_r[:], bsel_st[0:40, :].rearrange("p (a b) -> p a b", a=2 * CT))

            mv = pp.tile([128, CT, 2], F32, tag="mv")
            for k in range(CT):
                nc.vector.bn_aggr(mv[:, k, :], stat[k][:])

            # rstd = rsqrt(var+eps): ACT sqrt + DVE recip + one Newton step
            veps = pp.tile([128, CT], F32, tag="veps")
            nc.vector.tensor_scalar_add(veps[:], mv[:, :, 1], EPS)
            stdt = pp.tile([128, CT], F32, tag="stdt")
            nc.scalar.activation(stdt[:], veps[:], AF.Sqrt)
            r0 = pp.tile([128, CT], F32, tag="r0")
            nc.vector.reciprocal(r0[:], stdt[:])
            r2 = pp.tile([128, CT], F32, tag="r2")
            nc.vector.tensor_mul(r2[:], r0[:], r0[:])
            nc.vector.tensor_mul(r2[:], r2[:], veps[:])
            nc.vector.tensor_scalar(r2[:], r2[:], -0.5, 1.5, op0=OP.mult, op1=OP.add)
            rstd = pp.tile([128, CT], F32, tag="rstd")
            nc.vector.tensor_mul(rstd[:], r0[:], r2[:])
            # s = bn_w * rstd ; t = bn_b - mean * s
            s_t = pp.tile([128, CT], F32, tag="s_t")
            nc.vector.tensor_mul(s_t[:], rstd[:], bnw_t[:])
            t_t = pp.tile([128, CT], F32, tag="t_t")
            nc.vector.tensor_mul(t_t[:], mv[:, :, 0], s_t[:])
            nc.vector.tensor_sub(t_t[:], bnb_t[:], t_t[:])

            # ---- own image + normalize ------------------------------------
            ximg = [pp.tile([128, T], F32, tag=f"ximg{k}", name=f"ximg{k}") for k in range(CT)]
            xn = [pp.tile([128, T], F32R, tag=f"xn{k}", name=f"xn{k}") for k in range(CT)]
            for k in range(CT):
                nc.sync.dma_start(ximg[k][:], xall[0, 128 * k:128 * (k + 1), :])
                nc.gpsimd.tensor_scalar(
                    xn[k][:], ximg[k][:], s_t[:, k:k + 1], t_t[:, k:k + 1],
                    op0=OP.mult, op1=OP.add)

            # ---- phase Q: q/k/vT projections ------------------------------
            q_sb = [pp.tile([128, T], F32R, tag=f"q{m}", name=f"q{m}") for m in range(CT)]
            k_sb = [pp.tile([128, T], F32R, tag=f"k{m}", name=f"k{m}") for m in range(CT)]
            vt_sb = [pp.tile([128, NH, CH + 1], F32R, tag=f"vt{mt}", name=f"vt{mt}") for mt in range(8)]

            h_sb = [pp.tile([128, T], F32R, tag=f"h{m}", name=f"h{m}") for m in range(CT)]
            av_sb = [pp.tile([128, 512], F32, tag=f"avsb{i}", name=f"avsb{i}")
                     for i in range(NH)]
            rec_raw = pp.tile([40, 512], F32, tag="rec_raw")
            dstash = pp.tile([97, 4 * 512], F32, tag="dstash")
            rec_t = pp.tile([40, 512], F32R, tag="rec_t")

            def emit_qk(m):
                # one m-tile of q_all (m<4) / k_all rows into q_sb/k_sb
                for n in range(2):
                    ps = psA.tile([128, 512], F32, tag="ps", name="qk_ps")
                    for kk in range(CT):
                        nc.tensor.matmul(
                            ps[:], wqk_r[kk][:, 128 * m:128 * (m + 1)],
                            xn[kk][:, 512 * n:512 * (n + 1)],
                            start=(kk == 0), stop=(kk == CT - 1))
                    if m < CT:
                        nc.scalar.activation(
                            q_sb[m][:, 512 * n:512 * (n + 1)], ps[:],
                            AF.Identity, bias=qb_t[:, m:m + 1], scale=0.125)
                    else:
                        nc.scalar.activation(
                            k_sb[m - CT][:, 512 * n:512 * (n + 1)], ps[:],
                            AF.Identity, bias=kb_t[:, m - CT:m - CT + 1], scale=1.0)

            def emit_vt(mt):
                # vT = xn^T @ wv -> [t, c'], bias via DVE, ones column for the
                # softmax denominator row
                ps = psA.tile([128, 512], F32, tag="ps", name="vt_ps")
                for kk in range(CT):
                    nc.tensor.matmul(
                        ps[:], xn[kk][:, 128 * mt:128 * (mt + 1)], wv_r[kk][:],
                        start=(kk == 0), stop=(kk == CT - 1))
                nc.vector.tensor_add(
                    vt_sb[mt][:, :, 0:CH],
                    ps[:].rearrange("p (h c) -> p h c", h=NH),
                    vbb_t[:].rearrange("p (h c) -> p h c", h=NH))
                nc.vector.tensor_scalar(
                    vt_sb[mt][:, :, CH:CH + 1], vbb_t[:, 0:NH].unsqueeze(-1),
                    0.0, 1.0, op0=OP.mult, op1=OP.add)

            # ---- phases Q+A interleaved ------------------------------------
            # Attention for head pair g starts as soon as q/k tile g and the
            # vT tiles exist; qk tiles for later pairs are produced while the
            # ACT engine runs the (bottleneck) exp stream. Heads 2g / 2g+1 are
            # row-packed: their K=64 wT matmuls occupy distinct PE row groups
            # and run concurrently.
            for m in range(2 * CT):
                emit_qk(m)
            for mt in range(8):
                emit_vt(mt)

            groups = [(g, tcx) for g in range(4) for tcx in range(2)]

            def emit_wt(g, tcx, j):
                # [:, 0:512] head 2g, [:, 512:1024] head 2g+1
                wt = psW.tile([128, T], F32, tag="wt")
                for par in range(2):
                    nc.tensor.matmul(
                        wt[:, 512 * par:512 * (par + 1)],
                        k_sb[g][64 * par:64 * par + CH, 128 * j:128 * (j + 1)],
                        q_sb[g][64 * par:64 * par + CH, 512 * tcx:512 * (tcx + 1)],
                        start=True, stop=True)
                return wt

            wt_cur = emit_wt(0, 0, 0)
            for gi, (g, tcx) in enumerate(groups):
                av_e = psA.tile([CH + 1, 512], F32, tag="ps", name="av_e")
                av_o = psA.tile([CH + 1, 512], F32, tag="ps", name="av_o")
                for j in range(8):
                    # 1-ahead wT so the PE never waits on the exp stream
                    if j < 7:
                        wt_next = emit_wt(g, tcx, j + 1)
                    elif gi + 1 < len(groups):
                        g2, t2 = groups[gi + 1]
                        wt_next = emit_wt(g2, t2, 0)
                    else:
                        wt_next = None
                    e_t = ep.tile([128, T], F32R, tag="e")
                    nc.scalar.activation(e_t[:], wt_cur[:], AF.Exp)
                    nc.tensor.matmul(
                        av_e[:], vt_sb[j][:, 2 * g, :], e_t[:, 0:512],
                        start=(j == 0), stop=(j == 7), skip_group_check=True)
                    nc.tensor.matmul(
                        av_o[:], vt_sb[j][:, 2 * g + 1, :], e_t[:, 512:1024],
                        start=(j == 0), stop=(j == 7), skip_group_check=True)
                    wt_cur = wt_next
                pair = av_sb[g * 2 + tcx]
                for par, avp in ((0, av_e), (1, av_o)):
                    i = (2 * g + par) * 2 + tcx
                    nc.vector.tensor_copy(pair[64 * par:64 * par + CH, :],
                                          avp[0:CH, :])
                    nc.vector.tensor_copy(
                        dstash[32 * (i % 4):32 * (i % 4) + 1,
                               512 * (i // 4):512 * (i // 4) + 512],
                        avp[CH:CH + 1, :])
                for par in range(2):
                    i = (2 * g + par) * 2 + tcx
                    l, half = i % 8, i // 8
                    nc.sync.dma_start(
                        rec_raw[32 * half + l:32 * half + l + 1, :],
                        dstash[32 * (i % 4):32 * (i % 4) + 1,
                               512 * (i // 4):512 * (i // 4) + 512])

            # softmax division: batched reciprocals, selector-matmul broadcast,
            # DVE multiply straight off the broadcast PSUM
            for half in range(2):
                pbase = 32 * half
                with nc.allow_low_precision(reason="f32r softmax denom"):
                    nc.vector.reciprocal(rec_t[pbase:pbase + 8, :],
                                         rec_raw[pbase:pbase + 8, :])
            for g in range(4):
                for tcy in range(2):
                    pbase = 32 * (g // 2)
                    bc = psA.tile([128, 512], F32, tag="ps", name="bc")
                    nc.tensor.matmul(
                        bc[:], bsel_r[pbase:pbase + 8, g * 2 + tcy, :],
                        rec_t[pbase:pbase + 8, :], start=True, stop=True)
                    nc.vector.tensor_mul(
                        h_sb[g][:, 512 * tcy:512 * (tcy + 1)],
                        av_sb[g * 2 + tcy][:], bc[:])


            # ---- phase P: projection + residual ---------------------------
            for m in range(CT):
                for n in range(2):
                    ps = psA.tile([128, 512], F32, tag="ps")
                    for kk in range(CT):
                        nc.tensor.matmul(
                            ps[:], pw_r[kk][:, 128 * m:128 * (m + 1)],
                            h_sb[kk][:, 512 * n:512 * (n + 1)],
                            start=(kk == 0), stop=(kk == CT - 1))
                    ot = op_pool.tile([128, 512], F32, tag="ot")
                    nc.vector.scalar_tensor_tensor(
                        ot[:], ps[:], pb_t[:, m:m + 1],
                        ximg[m][:, 512 * n:512 * (n + 1)],
                        op0=OP.add, op1=OP.add)
                    nc.sync.dma_start(
                        out[128 * m:128 * (m + 1), 512 * n:512 * (n + 1)], ot[:])

    if split:
        split_sync_waits(nc)
    return nc


def _make_bsel():
    """Selector for broadcasting 1/D rows: bsel[k, g*2+tc, m] = 1 iff
    k == (2g + m//64)*2 + tc, so ones^T-style matmul replicates the right
    reciprocal row across each head's 64 output partitions."""
    sel = np.zeros((40, 2 * CT, 128), dtype=np.float32)
    for g in range(4):
        half = g // 2
        for tcx in range(2):
            for m in range(128):
                l = (2 * (g % 2) + m // 64) * 2 + tcx
                sel[32 * half + l, g * 2 + tcx, m] = 1.0
    return sel


def prep_inputs(x, bn_w, bn_b, qkv_w, qkv_b, proj_w, proj_b):
    """Host-side reshapes/permutations (no heavy compute)."""
    xall = np.ascontiguousarray(x.reshape(B, C, T))
    # reference head split: qkv.reshape(b*NH, 3*ch, T) -> head h gets rows
    # [192h,192h+64) = q, [192h+64,192h+128) = k, [192h+128,192h+192) = v
    hh = np.arange(NH)[:, None] * 3 * CH + np.arange(CH)[None, :]
    q_idx = hh.ravel()
    k_idx = (hh + CH).ravel()
    v_idx = (hh + 2 * CH).ravel()
    scale2 = 1.0 / np.sqrt(CH)  # folded into q (covers both q and k scales)

    common = {
        "wqkT": np.ascontiguousarray(qkv_w[np.r_[q_idx, k_idx]].T),
        "wvT": np.ascontiguousarray(qkv_w[v_idx].T),
        "pwT": np.ascontiguousarray(proj_w.T),
        "qb": np.ascontiguousarray((qkv_b[q_idx] * scale2).reshape(CT, 128).T),
        "kb": np.ascontiguousarray(qkv_b[k_idx].reshape(CT, 128).T),
        "vbb": np.ascontiguousarray(np.tile(qkv_b[v_idx][None, :], (128, 1))),
        "pb": np.ascontiguousarray(proj_b.reshape(CT, 128).T),
        "bnw": np.ascontiguousarray(bn_w.reshape(CT, 128).T),
        "bnb": np.ascontiguousarray(bn_b.reshape(CT, 128).T),
        "bsel": _make_bsel(),
    }
    in_maps = []
    for i in range(N_CORES):
        m = dict(common)
        m["xall"] = np.ascontiguousarray(np.roll(xall, -i, axis=0))
        in_maps.append(m)
    return in_maps


_NC_CACHE = {}


def _get_nc():
    if "nc" not in _NC_CACHE:
        _NC_CACHE["nc"] = build_nc()
    return _NC_CACHE["nc"]


def kernel(x, bn_w, bn_b, qkv_w, qkv_b, proj_w, proj_b, _trace=False):
    in_maps = prep_inputs(x, bn_w, bn_b, qkv_w, qkv_b, proj_w, proj_b)
    nc = _get_nc()
    res = run_bass_kernel_spmd(nc, in_maps, list(range(N_CORES)), trace=_trace)
    outs = np.stack([res.results[i]["out"] for i in range(N_CORES)], axis=0)
    full = outs.reshape(B, C, 32, 32).astype(np.float32)
    if _trace:
        kernel.last_result = res
    return full


# revision 26
# speedup vs baseline: 1.2262x; 1.2262x over previous
"""AttentionBlock (BatchNorm + 8-head self-attention + proj + residual) on 8 TRN2
NeuronCores, data-parallel over the batch of 8 images.

Reference computation (per batch of 8 images, c=512, T=32*32=1024):
  xf = x.reshape(b, c, T)
  mean/var over (b, T) per channel  (BatchNorm1d training mode, biased var)
  xn = (xf - mean) * rsqrt(var + eps) * bn_w + bn_b
  qkv = qkv_w @ xn + qkv_b          (per image; heads interleaved 192-row blocks)
  per head: softmax((q*s)^T (k*s)) @ v^T   with s = ch^-0.25
  out = xf + proj_w @ h + proj_b

Sharding: each core computes ONE image end-to-end. The only cross-image
coupling is the BN statistics; each core redundantly computes the global
stats from the full x (streamed tile-by-tile through bn_stats/bn_aggr).
Each core's input "xall" is rolled so its own image is xall[0] (stats are
order-invariant), keeping the program SPMD-identical across cores.

Matmuls run in float32r (full fp32 storage; PE rounds to a reduced internal
mantissa) which streams at 1 column/cycle for N>=256 -- 4x faster than fp32
with ~1e-5 relative error (measured on hw).
"""

import numpy as np

import concourse.bass as bass
import concourse.mybir as mybir
from concourse.bass_utils import run_bass_kernel_spmd
from concourse.tile import TileContext

F32 = mybir.dt.float32
F32R = mybir.dt.float32r
AF = mybir.ActivationFunctionType
OP = mybir.AluOpType

B = 8          # batch (== n cores)
C = 512        # channels
T = 1024       # sequence length (32*32)
NH = 8         # heads
CH = C // NH   # head dim = 64
CT = C // 128  # channel partition tiles = 4
EPS = 1e-5
N_CORES = 8


def split_sync_waits(nc, max_waits=1):
    """This walrus build rejects instructions carrying more than one sem wait
    ("Too many sync wait commands"); move overflow waits onto preceding NoOps
    on the same engine (per-engine program order preserves semantics)."""
    n_new = 0
    for f in nc.m.functions:
        for blk in f.blocks:
            insts = blk.instructions
            i = 0
            while i < len(insts):
                ins = insts[i]
                si = ins.sync_info
                if si is not None and len(si.on_wait) > max_waits:
                    waits = list(si.on_wait)
                    keep, rest = waits[:max_waits], waits[max_waits:]
                    ins.sync_info = type(si)(on_wait=keep, on_update=list(si.on_update))
                    while rest:
                        chunk, rest = rest[:max_waits], rest[max_waits:]
                        nop = mybir.InstNoOp(name=f"waitsplit_{n_new}", ins=[], outs=[])
                        n_new += 1
                        nop.engine = ins.engine
                        nop.sync_info = type(si)(on_wait=chunk, on_update=[])
                        insts.insert(i, nop)
                        i += 1
                i += 1
    return n_new


def build_nc(split=True, lookahead=True):
    nc = bass.Bass()

    xall = nc.declare_dram_parameter("xall", [B, C, T], F32, isOutput=False)
    wqkT = nc.declare_dram_parameter("wqkT", [C, 2 * C], F32, isOutput=False)
    wvT = nc.declare_dram_parameter("wvT", [C, C], F32, isOutput=False)
    pwT = nc.declare_dram_parameter("pwT", [C, C], F32, isOutput=False)
    qb = nc.declare_dram_parameter("qb", [128, CT], F32, isOutput=False)
    kb = nc.declare_dram_parameter("kb", [128, CT], F32, isOutput=False)
    vbb = nc.declare_dram_parameter("vbb", [128, C], F32, isOutput=False)
    pb = nc.declare_dram_parameter("pb", [128, CT], F32, isOutput=False)
    bnw = nc.declare_dram_parameter("bnw", [128, CT], F32, isOutput=False)
    bnb = nc.declare_dram_parameter("bnb", [128, CT], F32, isOutput=False)
    bsel = nc.declare_dram_parameter("bsel", [40, 2 * CT, 128], F32, isOutput=False)
    out = nc.declare_dram_parameter("out", [C, T], F32, isOutput=True)

    with TileContext(nc) as tc:
        with (
            tc.tile_pool(name="persist", bufs=1) as pp,
            tc.tile_pool(name="stage", bufs=3) as sp,
            tc.tile_pool(name="epool", bufs=2) as ep,
            tc.tile_pool(name="small", bufs=2) as smp,
            tc.tile_pool(name="opool", bufs=2) as op_pool,
            tc.tile_pool(name="psA", bufs=4, space="PSUM") as psA,
            tc.tile_pool(name="psW", bufs=2, space="PSUM") as psW,
        ):
            # ---- constants -------------------------------------------------
            qb_t = pp.tile([128, CT], F32, tag="qb")
            kb_t = pp.tile([128, CT], F32, tag="kb")
            vbb_t = pp.tile([128, C], F32, tag="vbb")
            pb_t = pp.tile([128, CT], F32, tag="pb")
            bnw_t = pp.tile([128, CT], F32, tag="bnw")
            bnb_t = pp.tile([128, CT], F32, tag="bnb")

            # ---- phase S: BN statistics over all 8 images ------------------
            # Emitted before weight loads so the 16 MiB stream starts at t=0;
            # two images per DMA via a strided 3D access pattern.
            stat = [pp.tile([128, 12 * B], F32, tag=f"stat{k}", name=f"stat{k}") for k in range(CT)]
            for j in range(0, B, 2):
                for k in range(CT):
                    xt = sp.tile([128, 2, T], F32, tag="xstat", bufs=2)
                    nc.sync.dma_start(
                        xt[:], xall[j:j + 2, 128 * k:128 * (k + 1), :]
                        .rearrange("j p t -> p j t"))
                    for jj in range(2):
                        nc.vector.bn_stats(stat[k][:, 12 * (j + jj):12 * (j + jj) + 6],
                                           xt[:, jj, 0:512])
                        nc.vector.bn_stats(stat[k][:, 12 * (j + jj) + 6:12 * (j + jj) + 12],
                                           xt[:, jj, 512:1024])

            nc.sync.dma_start(qb_t[:], qb[:])
            nc.sync.dma_start(kb_t[:], kb[:])
            nc.sync.dma_start(vbb_t[:], vbb[:])
            nc.sync.dma_start(pb_t[:], pb[:])
            nc.sync.dma_start(bnw_t[:], bnw[:])
            nc.sync.dma_start(bnb_t[:], bnb[:])
            ones64 = pp.tile([1, CH], F32R, tag="ones64")
            nc.vector.tensor_scalar(ones64[:], vbb_t[0:1, 0:CH], 0.0, 1.0,
                                    op0=OP.mult, op1=OP.add)

            # ---- weights: DMA then round to f32r on gpsimd -----------------
            wqk_r = [pp.tile([128, 2 * C], F32R, tag=f"wqk{k}", name=f"wqk{k}") for k in range(CT)]
            wv_r = [pp.tile([128, C], F32R, tag=f"wv{k}", name=f"wv{k}") for k in range(CT)]
            pw_r = [pp.tile([128, C], F32R, tag=f"pw{k}", name=f"pw{k}") for k in range(CT)]
            for k in range(CT):
                st = sp.tile([128, T], F32, tag="wst", bufs=2, name="wst1")
                nc.sync.dma_start(st[:], wqkT[128 * k:128 * (k + 1), :])
                nc.vector.tensor_copy(wqk_r[k][:], st[:])
            for k in range(CT):
                st = sp.tile([128, T], F32, tag="wst", bufs=2, name="wst2")
                nc.sync.dma_start(st[:, 0:C], wvT[128 * k:128 * (k + 1), :])
                nc.sync.dma_start(st[:, C:T], pwT[128 * k:128 * (k + 1), :])
                nc.gpsimd.tensor_copy(wv_r[k][:], st[:, 0:C])
                nc.gpsimd.tensor_copy(pw_r[k][:], st[:, C:T])

            bsel_r = pp.tile([40, 2 * CT, 128], F32R, tag="bsel_r")
            bsel_st = sp.tile([128, T], F32, tag="wst", bufs=2, name="bselst")
            nc.sync.dma_start(bsel_st[0:40, :], bsel[:].rearrange("p a b -> p (a b)"))
            nc.vector.tensor_copy(
                bsel_r[:], bsel_st[0:40, :].rearrange("p (a b) -> p a b", a=2 * CT))

            mv = pp.tile([128, CT, 2], F32, tag="mv")
            for k in range(CT):
                nc.vector.bn_aggr(mv[:, k, :], stat[k][:])

            # rstd = rsqrt(var+eps): ACT sqrt + DVE recip + one Newton step
            veps = pp.tile([128, CT], F32, tag="veps")
            nc.vector.tensor_scalar_add(veps[:], mv[:, :, 1], EPS)
            stdt = pp.tile([128, CT], F32, tag="stdt")
            nc.scalar.activation(stdt[:], veps[:], AF.Sqrt)
            r0 = pp.tile([128, CT], F32, tag="r0")
            nc.vector.reciprocal(r0[:], stdt[:])
            r2 = pp.tile([128, CT], F32, tag="r2")
            nc.vector.tensor_mul(r2[:], r0[:], r0[:])
            nc.vector.tensor_mul(r2[:], r2[:], veps[:])
            nc.vector.tensor_scalar(r2[:], r2[:], -0.5, 1.5, op0=OP.mult, op1=OP.add)
            rstd = pp.tile([128, CT], F32, tag="rstd")
            nc.vector.tensor_mul(rstd[:], r0[:], r2[:])
            # s = bn_w * rstd ; t = bn_b - mean * s
            s_t = pp.tile([128, CT], F32, tag="s_t")
            nc.vector.tensor_mul(s_t[:], rstd[:], bnw_t[:])
            t_t = pp.tile([128, CT], F32, tag="t_t")
            nc.vector.tensor_mul(t_t[:], mv[:, :, 0], s_t[:])
            nc.vector.tensor_sub(t_t[:], bnb_t[:], t_t[:])

            # ---- own image + normalize ------------------------------------
            ximg = [pp.tile([128, T], F32, tag=f"ximg{k}", name=f"ximg{k}") for k in range(CT)]
            xn = [pp.tile([128, T], F32R, tag=f"xn{k}", name=f"xn{k}") for k in range(CT)]
            for k in range(CT):
                nc.sync.dma_start(ximg[k][:], xall[0, 128 * k:128 * (k + 1), :])
                nc.gpsimd.tensor_scalar(
                    xn[k][:], ximg[k][:], s_t[:, k:k + 1], t_t[:, k:k + 1],
                    op0=OP.mult, op1=OP.add)

            # ---- phase Q: q/k/vT projections ------------------------------
            q_sb = [pp.tile([128, T], F32R, tag=f"q{m}", name=f"q{m}") for m in range(CT)]
            k_sb = [pp.tile([128, T], F32R, tag=f"k{m}", name=f"k{m}") for m in range(CT)]
            vt_sb = [pp.tile([128, NH, CH + 1], F32R, tag=f"vt{mt}", name=f"vt{mt}") for mt in range(8)]

            h_sb = [pp.tile([128, T], F32R, tag=f"h{m}", name=f"h{m}") for m in range(CT)]
            av_sb = [pp.tile([128, 512], F32, tag=f"avsb{i}", name=f"avsb{i}")
                     for i in range(NH)]
            rec_raw = pp.tile([40, 512], F32, tag="rec_raw")
            dstash = pp.tile([97, 4 * 512], F32, tag="dstash")
            rec_t = pp.tile([40, 512], F32R, tag="rec_t")

            def emit_qk(m):
                # one m-tile of q_all (m<4) / k_all rows into q_sb/k_sb
                for n in range(2):
                    ps = psA.tile([128, 512], F32, tag="ps", name="qk_ps")
                    for kk in range(CT):
                        nc.tensor.matmul(
                            ps[:], wqk_r[kk][:, 128 * m:128 * (m + 1)],
                            xn[kk][:, 512 * n:512 * (n + 1)],
                            start=(kk == 0), stop=(kk == CT - 1))
                    if m < CT:
                        nc.scalar.activation(
                            q_sb[m][:, 512 * n:512 * (n + 1)], ps[:],
                            AF.Identity, bias=qb_t[:, m:m + 1], scale=0.125)
                    else:
                        nc.scalar.activation(
                            k_sb[m - CT][:, 512 * n:512 * (n + 1)], ps[:],
                            AF.Identity, bias=kb_t[:, m - CT:m - CT + 1], scale=1.0)

            def emit_vt(mt):
                # vT = xn^T @ wv -> [t, c'], bias via DVE, ones column for the
                # softmax denominator row
                ps = psA.tile([128, 512], F32, tag="ps", name="vt_ps")
                for kk in range(CT):
                    nc.tensor.matmul(
                        ps[:], xn[kk][:, 128 * mt:128 * (mt + 1)], wv_r[kk][:],
                        start=(kk == 0), stop=(kk == CT - 1))
                nc.vector.tensor_add(
                    vt_sb[mt][:, :, 0:CH],
                    ps[:].rearrange("p (h c) -> p h c", h=NH),
                    vbb_t[:].rearrange("p (h c) -> p h c", h=NH))
                nc.vector.tensor_scalar(
                    vt_sb[mt][:, :, CH:CH + 1], vbb_t[:, 0:NH].unsqueeze(-1),
                    0.0, 1.0, op0=OP.mult, op1=OP.add)

            # ---- phases Q+A interleaved ------------------------------------
            # Attention for head pair g starts as soon as q/k tile g and the
            # vT tiles exist; qk tiles for later pairs are produced while the
            # ACT engine runs the (bottleneck) exp stream. Heads 2g / 2g+1 are
            # row-packed: their K=64 wT matmuls occupy distinct PE row groups
            # and run concurrently.
            for m in range(2 * CT):
                emit_qk(m)
            for mt in range(8):
                emit_vt(mt)

            groups = [(g, tcx) for g in range(4) for tcx in range(2)]

            def emit_wt(g, tcx, j):
                # [:, 0:512] head 2g, [:, 512:1024] head 2g+1
                wt = psW.tile([128, T], F32, tag="wt")
                for par in range(2):
                    nc.tensor.matmul(
                        wt[:, 512 * par:512 * (par + 1)],
                        k_sb[g][64 * par:64 * par + CH, 128 * j:128 * (j + 1)],
                        q_sb[g][64 * par:64 * par + CH, 512 * tcx:512 * (tcx + 1)],
                        start=True, stop=True)
                return wt

            wt_cur = emit_wt(0, 0, 0) if lookahead else None
            for gi, (g, tcx) in enumerate(groups):
                if not lookahead:
                    wt_cur = emit_wt(g, tcx, 0)
                av_e = psA.tile([CH + 1, 512], F32, tag="ps", name="av_e")
                av_o = psA.tile([CH + 1, 512], F32, tag="ps", name="av_o")
                for j in range(8):
                    # 1-ahead wT so the PE never waits on the exp stream
                    if j < 7:
                        wt_next = emit_wt(g, tcx, j + 1)
                    elif lookahead and gi + 1 < len(groups):
                        g2, t2 = groups[gi + 1]
                        wt_next = emit_wt(g2, t2, 0)
                    else:
                        wt_next = None
                    e_t = ep.tile([128, T], F32R, tag="e")
                    nc.scalar.activation(e_t[:], wt_cur[:], AF.Exp)
                    nc.tensor.matmul(
                        av_e[:], vt_sb[j][:, 2 * g, :], e_t[:, 0:512],
                        start=(j == 0), stop=(j == 7), skip_group_check=True)
                    nc.tensor.matmul(
                        av_o[:], vt_sb[j][:, 2 * g + 1, :], e_t[:, 512:1024],
                        start=(j == 0), stop=(j == 7), skip_group_check=True)
                    wt_cur = wt_next
                pair = av_sb[g * 2 + tcx]
                for par, avp in ((0, av_e), (1, av_o)):
                    i = (2 * g + par) * 2 + tcx
                    nc.vector.tensor_copy(pair[64 * par:64 * par + CH, :],
                                          avp[0:CH, :])
                    nc.vector.tensor_copy(
                        dstash[32 * (i % 4):32 * (i % 4) + 1,
                               512 * (i // 4):512 * (i // 4) + 512],
                        avp[CH:CH + 1, :])
                for par in range(2):
                    i = (2 * g + par) * 2 + tcx
                    l, half = i % 8, i // 8
                    nc.sync.dma_start(
                        rec_raw[32 * half + l:32 * half + l + 1, :],
                        dstash[32 * (i % 4):32 * (i % 4) + 1,
                               512 * (i // 4):512 * (i // 4) + 512])

            # softmax division: batched reciprocals, selector-matmul broadcast,
            # DVE multiply straight off the broadcast PSUM
            for half in range(2):
                pbase = 32 * half
                with nc.allow_low_precision(reason="f32r softmax denom"):
                    nc.vector.reciprocal(rec_t[pbase:pbase + 8, :],
                                         rec_raw[pbase:pbase + 8, :])
            for g in range(4):
                for tcy in range(2):
                    pbase = 32 * (g // 2)
                    bc = psA.tile([128, 512], F32, tag="ps", name="bc")
                    nc.tensor.matmul(
                        bc[:], bsel_r[pbase:pbase + 8, g * 2 + tcy, :],
                        rec_t[pbase:pbase + 8, :], start=True, stop=True)
                    nc.vector.tensor_mul(
                        h_sb[g][:, 512 * tcy:512 * (tcy + 1)],
                        av_sb[g * 2 + tcy][:], bc[:])


            # ---- phase P: projection + residual ---------------------------
            for m in range(CT):
                for n in range(2):
                    ps = psA.tile([128, 512], F32, tag="ps")
                    for kk in range(CT):
                        nc.tensor.matmul(
                            ps[:], pw_r[kk][:, 128 * m:128 * (m + 1)],
                            h_sb[kk][:, 512 * n:512 * (n + 1)],
                            start=(kk == 0), stop=(kk == CT - 1))
                    ot = op_pool.tile([128, 512], F32, tag="ot")
                    nc.vector.scalar_tensor_tensor(
                        ot[:], ps[:], pb_t[:, m:m + 1],
                        ximg[m][:, 512 * n:512 * (n + 1)],
                        op0=OP.add, op1=OP.add)
                    nc.sync.dma_start(
                        out[128 * m:128 * (m + 1), 512 * n:512 * (n + 1)], ot[:])

    if split:
        split_sync_waits(nc)
    return nc


def _make_bsel():
    """Selector for broadcasting 1/D rows: bsel[k, g*2+tc, m] = 1 iff
    k == (2g + m//64)*2 + tc, so ones^T-style matmul replicates the right
    reciprocal row across each head's 64 output partitions."""
    sel = np.zeros((40, 2 * CT, 128), dtype=np.float32)
    for g in range(4):
        half = g // 2
        for tcx in range(2):
            for m in range(128):
                l = (2 * (g % 2) + m // 64) * 2 + tcx
                sel[32 * half + l, g * 2 + tcx, m] = 1.0
    return sel


def prep_inputs(x, bn_w, bn_b, qkv_w, qkv_b, proj_w, proj_b):
    """Host-side reshapes/permutations (no heavy compute)."""
    xall = np.ascontiguousarray(x.reshape(B, C, T))
    # reference head split: qkv.reshape(b*NH, 3*ch, T) -> head h gets rows
    # [192h,192h+64) = q, [192h+64,192h+128) = k, [192h+128,192h+192) = v
    hh = np.arange(NH)[:, None] * 3 * CH + np.arange(CH)[None, :]
    q_idx = hh.ravel()
    k_idx = (hh + CH).ravel()
    v_idx = (hh + 2 * CH).ravel()
    scale2 = 1.0 / np.sqrt(CH)  # folded into q (covers both q and k scales)

    common = {
        "wqkT": np.ascontiguousarray(qkv_w[np.r_[q_idx, k_idx]].T),
        "wvT": np.ascontiguousarray(qkv_w[v_idx].T),
        "pwT": np.ascontiguousarray(proj_w.T),
        "qb": np.ascontiguousarray((qkv_b[q_idx] * scale2).reshape(CT, 128).T),
        "kb": np.ascontiguousarray(qkv_b[k_idx].reshape(CT, 128).T),
        "vbb": np.ascontiguousarray(np.tile(qkv_b[v_idx][None, :], (128, 1))),
        "pb": np.ascontiguousarray(proj_b.reshape(CT, 128).T),
        "bnw": np.ascontiguousarray(bn_w.reshape(CT, 128).T),
        "bnb": np.ascontiguousarray(bn_b.reshape(CT, 128).T),
        "bsel": _make_bsel(),
    }
    in_maps = []
    for i in range(N_CORES):
        m = dict(common)
        m["xall"] = np.ascontiguousarray(np.roll(xall, -i, axis=0))
        in_maps.append(m)
    return in_maps


_NC_CACHE = {}


def _get_nc():
    if "nc" not in _NC_CACHE:
        _NC_CACHE["nc"] = build_nc()
    return _NC_CACHE["nc"]


def kernel(x, bn_w, bn_b, qkv_w, qkv_b, proj_w, proj_b, _trace=False):
    in_maps = prep_inputs(x, bn_w, bn_b, qkv_w, qkv_b, proj_w, proj_b)
    nc = _get_nc()
    res = run_bass_kernel_spmd(nc, in_maps, list(range(N_CORES)), trace=_trace)
    outs = np.stack([res.results[i]["out"] for i in range(N_CORES)], axis=0)
    full = outs.reshape(B, C, 32, 32).astype(np.float32)
    if _trace:
        kernel.last_result = res
    return full
